# revision 7
# baseline (speedup 1.0000x reference)
"""Distributed sparse MoE (top-1 routing) kernel for 8 TRN2 NeuronCores.

Strategy (expert-parallel, sliced AllToAll dispatch):
  - Tokens sharded 1024/core. Each core routes its slice (fp32 PE logits ->
    argmax/gate, matching the reference bit-for-bit).
  - Slot assignment on PE: one-hot(expert) rows fed through lower-triangular
    matmuls give each token its rank within its (src core, expert) bin of
    capacity 256; rows [x bf16 | gate f32 | global id f32] are indirect-DMA
    scattered from SBUF into per-round AllToAll inputs (round 0: bin slots
    0-127, round 1: 128-255). Global ids go to a metadata buffer
    (wrap16-within-shard layout, sentinel-initialized).
  - Three AllToAlls: 8KB metadata, then the two 2.1MB payload rounds.
    Receiver compaction (valid-mask -> sparse_gather; caps 1024/256) runs
    during the payload flights; round-0 GEMM overlaps the round-1 A2A.
  - Per 128-token tile: indirect-gather payload rows, PE-transpose, bf16
    expert GEMM with fp32 accumulate, (out + bias) * gate at eviction.
  - Outputs: dense rows [1280, H], per-round slot maps, and the metadata;
    the host places rows at meta[slot]. Junk rows carry sentinel slots and
    are dropped.
"""

import sys

sys.path.insert(0, "/opt/trn_rl_repo")

import ml_dtypes
import numpy as np

import concourse.bass as bass
import concourse.mybir as mybir
import concourse.tile as tile
from concourse import bacc
from concourse.bass_utils import run_bass_kernel_spmd
from concourse.masks import make_identity, make_upper_triangular

F32 = mybir.dt.float32
BF16 = mybir.dt.bfloat16
I32 = mybir.dt.int32
U32 = mybir.dt.uint32

N_CORES = 8
B, S, H, E = 4, 2048, 1024, 8
T = B * S                # 8192 tokens
TPC = T // N_CORES       # 1024 tokens per core slice
TILES = TPC // 128       # 8 token tiles per slice
HC = H // 128            # 8 contraction chunks
BINCAP = 256             # per-(src,dst) bin capacity (observed max 172)
RCAP0 = 1024             # round-0 receiver capacity (slots 0-127; exact bound)
RCAP1 = 256              # round-1 receiver capacity (observed max ~91)
RT0 = RCAP0 // 128       # 8 round-0 GEMM tiles
RT1 = RCAP1 // 128       # 2 round-1 GEMM tiles
NSLOT = N_CORES * BINCAP  # 2048 logical slots
NP = N_CORES * 128       # 1024 rows per payload round
W = 1032                 # bf16 row: 1024 x + gate(f32) + gid(f32)
GCOL = 512               # f32-view column of gate
ICOL = 513               # f32-view column of gid
SENT = 65536.0           # sentinel for empty slots / tails
OOB = 60000.0            # out-of-bounds scatter offset (skipped)
NHALF = 2                # 1024 output dims in 2 x 512 psum halves


def _body(tc, x, rw, rb, ew, eb, gid, erow, iota_r0, iota_r1, slots_r0,
          slots_r1, out_rows, out_gsel0, out_gsel1, out_meta):
    nc = tc.nc
    P = 128
    Exp = mybir.ActivationFunctionType.Exp
    rg = [list(range(N_CORES))]

    dram = tc.alloc_tile_pool(name="dram", bufs=1, space="DRAM")
    pay_in = [dram.tile([NP, W], BF16, name=f"pay_in{i}") for i in range(2)]
    pay_out = [dram.tile([NP, W], BF16, name=f"pay_out{i}") for i in range(2)]
    meta_in = dram.tile([NSLOT], F32)
    meta_out = dram.tile([NSLOT], F32)
    rflat0 = dram.tile([RCAP0], I32)
    rflat1 = dram.tile([RCAP1], I32)

    const = tc.alloc_tile_pool(name="const", bufs=1)
    ident = const.tile([P, P], F32)
    make_identity(nc, ident)
    ones = const.tile([P, P], F32)
    nc.vector.memset(ones[:], 1.0)
    triu = const.tile([P, P], F32)
    make_upper_triangular(nc, triu[:], val=1.0, diag=True)
    identb = const.tile([P, P], BF16)
    nc.vector.tensor_copy(identb[:], ident[:])

    rw_sb = const.tile([P, HC, E], F32)
    nc.sync.dma_start(rw_sb[:], rw.rearrange("(c p) e -> p c e", p=P))
    rb_sb = const.tile([1, E], F32)
    nc.sync.dma_start(rb_sb[:], rb[:])
    rb_rep = const.tile([P, E], F32)
    nc.gpsimd.partition_broadcast(rb_rep[:], rb_sb[:])
    erow_sb = const.tile([1, E], F32)
    nc.sync.dma_start(erow_sb[:], erow[:])
    erow_rep = const.tile([P, E], F32)
    nc.gpsimd.partition_broadcast(erow_rep[:], erow_sb[:])
    gid_sb = const.tile([P, TILES], F32)
    nc.sync.dma_start(gid_sb[:], gid[:])
    iota0_sb = const.tile([16, NSLOT // 16], F32)
    nc.sync.dma_start(iota0_sb[:], iota_r0[:])
    iota1_sb = const.tile([16, NSLOT // 16], F32)
    nc.sync.dma_start(iota1_sb[:], iota_r1[:])
    slots0_sb = const.tile([16, RCAP0 // 16], F32)
    nc.sync.dma_start(slots0_sb[:], slots_r0[:])
    slots1_sb = const.tile([16, RCAP1 // 16], F32)
    nc.sync.dma_start(slots1_sb[:], slots_r1[:])

    # meta_in := sentinel everywhere (slots no scatter writes stay invalid)
    sent16 = const.tile([16, NSLOT // 16], F32)
    nc.vector.memset(sent16[:], SENT)
    nc.sync.dma_start(meta_in[:].rearrange("(p f) -> p f", p=16), sent16[:])

    # ---- Phase A: router + slot assignment + scatter into A2A inputs ----
    ohist = [const.tile([P, E], F32, name=f"ohist{i}") for i in range(TILES)]
    with tc.tile_pool(name="workA", bufs=6) as workA, tc.tile_pool(
        name="psumA", bufs=2, space="PSUM"
    ) as psumA, tc.tile_pool(name="psumL", bufs=2, space="PSUM") as psumL, \
        tc.tile_pool(name="psumP", bufs=2, space="PSUM") as psumP:
        for t in range(TILES):
            xt = workA.tile([P, H], F32, tag="xt")
            nc.sync.dma_start(xt[:], x[t * P : (t + 1) * P, :])
            xT = workA.tile([P, H], F32, tag="xT")
            pt = psumA.tile([P, H], F32, tag="pt")
            for c in range(HC):
                nc.tensor.transpose(
                    pt[:, c * P : (c + 1) * P], xt[:, c * P : (c + 1) * P], ident[:]
                )
            if t % 2 == 0:
                nc.scalar.copy(xT[:], pt[:])
            else:
                nc.vector.tensor_copy(xT[:], pt[:])
            lp = psumL.tile([P, E], F32, tag="lp")
            for c in range(HC):
                nc.tensor.matmul(
                    lp[:],
                    lhsT=xT[:, c * P : (c + 1) * P],
                    rhs=rw_sb[:, c, :],
                    start=(c == 0),
                    stop=(c == HC - 1),
                )
            logits = workA.tile([P, E], F32, tag="logits")
            nc.vector.tensor_tensor(logits[:], lp[:], rb_rep[:], mybir.AluOpType.add)
            negmax = workA.tile([P, 1], F32, tag="negmax")
            nc.vector.reduce_max(
                negmax[:], logits[:], mybir.AxisListType.X, negate=True
            )
            expd = workA.tile([P, E], F32, tag="expd")
            esum = workA.tile([P, 1], F32, tag="esum")
            nc.scalar.activation(
                expd[:], logits[:], Exp, bias=negmax[:], accum_out=esum[:]
            )
            gate = workA.tile([P, 1], F32, tag="gate")
            nc.vector.reciprocal(gate[:], esum[:])
            mx8 = workA.tile([P, 8], F32, tag="mx8")
            nc.vector.max(mx8[:], logits[:])
            mi = workA.tile([P, 8], U32, tag="mi")
            nc.vector.max_index(mi[:], mx8[:], logits[:])
            idxf = workA.tile([P, 1], F32, tag="idxf")
            nc.vector.tensor_copy(idxf[:], mi[:, 0:1])

            # one-hot of routed expert; rank via triangular matmul prefix
            nc.vector.tensor_scalar(
                ohist[t][:], erow_rep[:], idxf[:], None,
                op0=mybir.AluOpType.is_equal,
            )
            pfx = psumP.tile([P, E], F32, tag="pfx")
            for a in range(t + 1):
                nc.tensor.matmul(
                    pfx[:],
                    lhsT=ones[:] if a < t else triu[:],
                    rhs=ohist[a][:],
                    start=(a == 0),
                    stop=(a == t),
                )
            ranked = workA.tile([P, E], F32, tag="ranked")
            nc.vector.tensor_tensor(
                ranked[:], pfx[:], ohist[t][:], mybir.AluOpType.mult
            )
            rank = workA.tile([P, 1], F32, tag="rank")
            nc.vector.reduce_sum(rank[:], ranked[:], mybir.AxisListType.X)
            # slot-in-bin (0-based, clamped to bin capacity)
            sb = workA.tile([P, 1], F32, tag="sb")
            nc.vector.tensor_scalar(
                sb[:], rank[:], -1.0, float(BINCAP - 1),
                op0=mybir.AluOpType.add, op1=mybir.AluOpType.min,
            )
            # meta offset: e*256 + (sb%16)*16 + sb//16  (wrap16 within shard)
            slot = workA.tile([P, 1], F32, tag="slot")
            nc.vector.tensor_scalar(
                slot[:], idxf[:], float(BINCAP), sb[:],
                op0=mybir.AluOpType.mult, op1=mybir.AluOpType.add,
            )
            si = workA.tile([P, 1], I32, tag="si")
            nc.vector.tensor_copy(si[:], slot[:])
            sbi = workA.tile([P, 1], I32, tag="sbi")
            nc.vector.tensor_copy(sbi[:], sb[:])
            m16 = workA.tile([P, 1], I32, tag="m16")
            nc.vector.tensor_scalar(
                m16[:], sbi[:], 15, 4,
                op0=mybir.AluOpType.bitwise_and,
                op1=mybir.AluOpType.logical_shift_left,
            )
            d16 = workA.tile([P, 1], I32, tag="d16")
            nc.vector.tensor_scalar(
                d16[:], sbi[:], 4, None, op0=mybir.AluOpType.logical_shift_right
            )
            base = workA.tile([P, 1], I32, tag="base")
            nc.vector.tensor_tensor(base[:], si[:], sbi[:], mybir.AluOpType.subtract)
            qoff = workA.tile([P, 1], I32, tag="qoff")
            nc.vector.tensor_tensor(qoff[:], base[:], m16[:], mybir.AluOpType.add)
            nc.vector.tensor_tensor(qoff[:], qoff[:], d16[:], mybir.AluOpType.add)
            nc.gpsimd.indirect_dma_start(
                out=meta_in[:].rearrange("(n one) -> n one", one=1),
                out_offset=bass.IndirectOffsetOnAxis(ap=qoff[:], axis=0),
                in_=gid_sb[:, t : t + 1],
                in_offset=None,
                bounds_check=NSLOT - 1,
                oob_is_err=False,
            )

            # per-round payload offsets: e*128 + (sb % 128), OOB otherwise
            m0 = workA.tile([P, 1], F32, tag="m0")
            nc.vector.tensor_scalar(
                m0[:], sb[:], 128.0, None, op0=mybir.AluOpType.is_lt
            )
            tcom = workA.tile([P, 1], F32, tag="tcom")
            nc.vector.tensor_scalar(
                tcom[:], idxf[:], 128.0, sb[:],
                op0=mybir.AluOpType.mult, op1=mybir.AluOpType.add,
            )
            u0 = workA.tile([P, 1], F32, tag="u0")
            nc.vector.tensor_scalar(
                u0[:], m0[:], -OOB, OOB,
                op0=mybir.AluOpType.mult, op1=mybir.AluOpType.add,
            )
            o0f = workA.tile([P, 1], F32, tag="o0f")
            nc.vector.tensor_tensor(o0f[:], tcom[:], u0[:], mybir.AluOpType.add)
            u1 = workA.tile([P, 1], F32, tag="u1")
            nc.vector.tensor_scalar(
                u1[:], m0[:], OOB, -128.0,
                op0=mybir.AluOpType.mult, op1=mybir.AluOpType.add,
            )
            o1f = workA.tile([P, 1], F32, tag="o1f")
            nc.vector.tensor_tensor(o1f[:], tcom[:], u1[:], mybir.AluOpType.add)
            o0i = workA.tile([P, 1], I32, tag="o0i")
            nc.vector.tensor_copy(o0i[:], o0f[:])
            o1i = workA.tile([P, 1], I32, tag="o1i")
            nc.vector.tensor_copy(o1i[:], o1f[:])

            xs = workA.tile([P, W], BF16, tag="xs")
            if t % 2 == 0:
                nc.vector.tensor_copy(xs[:, 0:H], xt[:])
            else:
                nc.scalar.copy(xs[:, 0:H], xt[:])
            xsf = xs[:].bitcast(F32)
            nc.vector.tensor_copy(xsf[:, GCOL : GCOL + 1], gate[:])
            nc.vector.tensor_copy(xsf[:, ICOL : ICOL + 1], gid_sb[:, t : t + 1])

            nc.gpsimd.indirect_dma_start(
                out=pay_in[0][:],
                out_offset=bass.IndirectOffsetOnAxis(ap=o0i[:], axis=0),
                in_=xs[:],
                in_offset=None,
                bounds_check=NP - 1,
                oob_is_err=False,
            )
            nc.gpsimd.indirect_dma_start(
                out=pay_in[1][:],
                out_offset=bass.IndirectOffsetOnAxis(ap=o1i[:], axis=0),
                in_=xs[:],
                in_offset=None,
                bounds_check=NP - 1,
                oob_is_err=False,
            )

    # ---- Phase B: metadata A2A, then the two payload rounds ----
    nc.gpsimd.collective_compute(
        "AllToAll", mybir.AluOpType.bypass, replica_groups=rg,
        ins=[meta_in[:].opt()], outs=[meta_out[:].opt()])
    nc.gpsimd.collective_compute(
        "AllToAll", mybir.AluOpType.bypass, replica_groups=rg,
        ins=[pay_in[0][:].opt()], outs=[pay_out[0][:].opt()])
    nc.gpsimd.collective_compute(
        "AllToAll", mybir.AluOpType.bypass, replica_groups=rg,
        ins=[pay_in[1][:].opt()], outs=[pay_out[1][:].opt()])

    # expert weights are only needed by phase D; load after phase A issues
    w_sb = const.tile([P, HC, H], BF16)
    nc.sync.dma_start(w_sb[:], ew.rearrange("(c p) d -> p c d", p=P))
    eb_sb = const.tile([1, H], F32)
    nc.sync.dma_start(eb_sb[:], eb[:])
    b_rep = const.tile([P, H], F32)
    nc.gpsimd.partition_broadcast(b_rep[:], eb_sb[:])

    # ---- Phase C: receiver compaction from metadata (both rounds) ----
    sel = tc.alloc_tile_pool(name="sel", bufs=1)
    meta16 = sel.tile([16, NSLOT // 16], F32)
    nc.sync.dma_start(
        meta16[:].rearrange("p (e c) -> p e c", c=16),
        meta_out[:].rearrange("(e p c) -> p e c", p=16, c=16),
    )
    nc.sync.dma_start(
        out_meta[:].rearrange("(e p c) -> p e c", p=16, c=16),
        meta16[:].rearrange("p (e c) -> p e c", c=16),
    )
    vmask = sel.tile([16, NSLOT // 16], F32)
    nc.vector.tensor_scalar(
        vmask[:], meta16[:], float(T), None, op0=mybir.AluOpType.is_lt
    )
    ridx = []
    for r, (iota_sb, slots_sb, rcap, rflat, out_gsel) in enumerate([
        (iota0_sb, slots0_sb, RCAP0, rflat0, out_gsel0),
        (iota1_sb, slots1_sb, RCAP1, rflat1, out_gsel1),
    ]):
        val = sel.tile([16, NSLOT // 16], F32)
        nc.vector.tensor_tensor(val[:], iota_sb[:], vmask[:], mybir.AluOpType.mult)
        nc.vector.tensor_scalar_add(val[:], val[:], -1.0)
        rstage = sel.tile([16, rcap // 16], F32)
        rcnt = sel.tile([1, 1], U32)
        nc.gpsimd.sparse_gather(rstage[:], val[:], num_found=rcnt[:])
        rcntf = sel.tile([1, 1], F32)
        nc.vector.tensor_copy(rcntf[:], rcnt[:])
        rcnt16 = sel.tile([16, 1], F32)
        nc.gpsimd.partition_broadcast(rcnt16[:], rcntf[:])
        tailm = sel.tile([16, rcap // 16], F32)
        nc.vector.tensor_scalar(
            tailm[:], slots_sb[:], rcnt16[:], None, op0=mybir.AluOpType.is_lt
        )
        fixed = sel.tile([16, rcap // 16], F32)
        nc.vector.tensor_scalar_add(fixed[:], rstage[:], -SENT)
        nc.vector.tensor_tensor(fixed[:], fixed[:], tailm[:], mybir.AluOpType.mult)
        nc.vector.tensor_scalar_add(fixed[:], fixed[:], SENT)
        ri32 = sel.tile([16, rcap // 16], I32)
        nc.vector.tensor_copy(ri32[:], fixed[:])
        nc.sync.dma_start(rflat[:].rearrange("(f p) -> p f", p=16), ri32[:])
        nc.sync.dma_start(out_gsel[:].rearrange("(f p) -> p f", p=16), ri32[:])
        rx = sel.tile([P, rcap // P], I32, tag=f"ridx{r}")
        nc.sync.dma_start(rx[:], rflat[:].rearrange("(j p) -> p j", p=P))
        ridx.append(rx)

    # ---- Phase D: gather payload rows, expert GEMM, write dense rows ----
    with tc.tile_pool(name="workD", bufs=3) as workD, tc.tile_pool(
        name="gpool", bufs=3
    ) as gpool, tc.tile_pool(name="psumT", bufs=2, space="PSUM") as psumT, \
        tc.tile_pool(name="psumG", bufs=2, space="PSUM") as psumG:
        for j in range(RT0 + RT1):
            r, jj = (0, j) if j < RT0 else (1, j - RT0)
            gath = gpool.tile([P, W], BF16, tag="gath")
            nc.gpsimd.indirect_dma_start(
                out=gath[:],
                out_offset=None,
                in_=pay_out[r][:],
                in_offset=bass.IndirectOffsetOnAxis(
                    ap=ridx[r][:, jj : jj + 1], axis=0
                ),
                bounds_check=NP - 1,
                oob_is_err=False,
            )
            xTg = workD.tile([P, HC, P], BF16, tag="xTg")
            pt = psumT.tile([P, H], BF16, tag="pt")
            for c in range(HC):
                nc.tensor.transpose(
                    pt[:, c * P : (c + 1) * P], gath[:, c * P : (c + 1) * P], identb[:]
                )
            nc.scalar.copy(xTg[:].rearrange("p c d -> p (c d)"), pt[:])
            gate_g = gath[:].bitcast(F32)[:, GCOL : GCOL + 1]
            outj = workD.tile([P, H], F32, tag="outj")
            for h in range(NHALF):
                pg = psumG.tile([P, 512], F32, tag="pg")
                for c in range(HC):
                    nc.tensor.matmul(
                        pg[:],
                        lhsT=xTg[:, c, :],
                        rhs=w_sb[:, c, h * 512 : (h + 1) * 512],
                        start=(c == 0),
                        stop=(c == HC - 1),
                    )
                nc.vector.tensor_tensor(
                    outj[:, h * 512 : (h + 1) * 512],
                    pg[:],
                    b_rep[:, h * 512 : (h + 1) * 512],
                    mybir.AluOpType.add,
                )
                nc.vector.tensor_scalar_mul(
                    outj[:, h * 512 : (h + 1) * 512],
                    outj[:, h * 512 : (h + 1) * 512],
                    gate_g,
                )
            nc.sync.dma_start(out_rows[j * P : (j + 1) * P, :], outj[:])

    sel.release()
    const.release()
    dram.release()


def build_kernel():
    nc = bacc.Bacc(
        "TRN2",
        target_bir_lowering=False,
        debug=False,
        enable_asserts=True,
        num_devices=N_CORES,
    )
    x = nc.dram_tensor("x", [TPC, H], F32, kind="ExternalInput").ap()
    rw = nc.dram_tensor("router_w", [H, E], F32, kind="ExternalInput").ap()
    rb = nc.dram_tensor("router_b", [1, E], F32, kind="ExternalInput").ap()
    ew = nc.dram_tensor("expert_w", [H, H], BF16, kind="ExternalInput").ap()
    eb = nc.dram_tensor("expert_b", [1, H], F32, kind="ExternalInput").ap()
    gid = nc.dram_tensor("gid", [128, TILES], F32, kind="ExternalInput").ap()
    erow = nc.dram_tensor("erow", [1, E], F32, kind="ExternalInput").ap()
    iota_r0 = nc.dram_tensor(
        "iota_r0", [16, NSLOT // 16], F32, kind="ExternalInput"
    ).ap()
    iota_r1 = nc.dram_tensor(
        "iota_r1", [16, NSLOT // 16], F32, kind="ExternalInput"
    ).ap()
    slots_r0 = nc.dram_tensor(
        "slots_r0", [16, RCAP0 // 16], F32, kind="ExternalInput"
    ).ap()
    slots_r1 = nc.dram_tensor(
        "slots_r1", [16, RCAP1 // 16], F32, kind="ExternalInput"
    ).ap()
    out_rows = nc.dram_tensor(
        "out_rows", [RCAP0 + RCAP1, H], F32, kind="ExternalOutput"
    ).ap()
    out_gsel0 = nc.dram_tensor("out_gsel0", [RCAP0], I32, kind="ExternalOutput").ap()
    out_gsel1 = nc.dram_tensor("out_gsel1", [RCAP1], I32, kind="ExternalOutput").ap()
    out_meta = nc.dram_tensor("out_meta", [NSLOT], F32, kind="ExternalOutput").ap()

    with tile.TileContext(nc) as tc:
        _body(tc, x, rw, rb, ew, eb, gid, erow, iota_r0, iota_r1, slots_r0,
              slots_r1, out_rows, out_gsel0, out_gsel1, out_meta)
    nc.compile()
    return nc


_CACHE = {}


def _wrap16(vals):
    """Values laid out so element k sits at [k % 16, k // 16]."""
    a = np.asarray(vals, dtype=np.float32)
    return a.reshape(-1, 16).T.copy()


def kernel(x, router_w, router_b, expert_w, expert_b, **run_kwargs):
    x = np.ascontiguousarray(np.asarray(x, dtype=np.float32))
    router_w = np.ascontiguousarray(np.asarray(router_w, dtype=np.float32))
    router_b = np.ascontiguousarray(np.asarray(router_b, dtype=np.float32))
    expert_w = np.ascontiguousarray(np.asarray(expert_w, dtype=np.float32))
    expert_b = np.ascontiguousarray(np.asarray(expert_b, dtype=np.float32))

    hs = x.reshape(T, H)
    # per-slot values for the receiver selects: round-local payload row + 1
    # where slot g = e*256 + sb belongs to the round, else 0
    g = np.arange(NSLOT)
    e, sb = g // BINCAP, g % BINCAP
    i0 = np.where(sb < 128, e * 128 + sb + 1, 0).astype(np.float32)
    i1 = np.where(sb >= 128, e * 128 + (sb - 128) + 1, 0).astype(np.float32)
    iota_r0 = _wrap16(i0)
    iota_r1 = _wrap16(i1)
    slots_r0 = _wrap16(np.arange(RCAP0, dtype=np.float32))
    slots_r1 = _wrap16(np.arange(RCAP1, dtype=np.float32))
    erow = np.arange(E, dtype=np.float32).reshape(1, E)

    if "nc" not in _CACHE:
        _CACHE["nc"] = build_kernel()
    nc = _CACHE["nc"]

    in_maps = []
    for c in range(N_CORES):
        gid = (
            c * TPC
            + np.arange(TILES)[None, :] * 128
            + np.arange(128)[:, None]
        ).astype(np.float32)
        in_maps.append(
            {
                "x": hs[c * TPC : (c + 1) * TPC],
                "router_w": router_w,
                "router_b": router_b.reshape(1, E),
                "expert_w": expert_w[c].astype(ml_dtypes.bfloat16),
                "expert_b": expert_b[c].reshape(1, H),
                "gid": gid,
                "erow": erow,
                "iota_r0": iota_r0,
                "iota_r1": iota_r1,
                "slots_r0": slots_r0,
                "slots_r1": slots_r1,
            }
        )

    res = run_bass_kernel_spmd(nc, in_maps, core_ids=list(range(N_CORES)), **run_kwargs)
    full = np.zeros((T, H), dtype=np.float32)
    for r in res.results:
        meta = r["out_meta"]
        rows = r["out_rows"]
        for rnd, (gsel, base, off) in enumerate(
            [(r["out_gsel0"], 0, 0), (r["out_gsel1"], RCAP0, 128)]
        ):
            valid = (gsel >= 0) & (gsel < NP)
            rowid = np.nonzero(valid)[0]
            gv = gsel[valid].astype(np.int64)
            e, sb = gv // 128, gv % 128 + off
            # meta is wrap16-permuted within each 256-slot shard
            q = e * BINCAP + (sb % 16) * 16 + sb // 16
            gids = meta[q].astype(np.int64)
            inner = (gids >= 0) & (gids < T)
            full[gids[inner]] = rows[base + rowid[inner]]
    out = full.reshape(B, S, H)
    if run_kwargs:
        return out, res
    return out


# revision 10
# speedup vs baseline: 1.1500x; 1.1500x over previous
"""Distributed sparse MoE (top-1 routing) kernel for 8 TRN2 NeuronCores.

Strategy (expert-parallel, sliced AllToAll dispatch):
  - Tokens sharded 1024/core. Each core routes its slice (fp32 PE logits ->
    argmax/gate, matching the reference bit-for-bit).
  - Slot assignment on PE: one-hot(expert) rows fed through lower-triangular
    matmuls give each token its rank within its (src core, expert) bin of
    capacity 256; rows [x bf16 | gate f32 | global id f32] are indirect-DMA
    scattered from SBUF into per-round AllToAll inputs (round 0: bin slots
    0-127, round 1: 128-255). Global ids go to a metadata buffer
    (wrap16-within-shard layout, sentinel-initialized).
  - Three AllToAlls: 8KB metadata, then the two 2.1MB payload rounds.
    Receiver compaction (valid-mask -> sparse_gather; caps 1024/256) runs
    during the payload flights; round-0 GEMM overlaps the round-1 A2A.
  - Per 128-token tile: indirect-gather payload rows, PE-transpose, bf16
    expert GEMM with fp32 accumulate, (out + bias) * gate at eviction.
  - Outputs: dense rows [1280, H], per-round slot maps, and the metadata;
    the host places rows at meta[slot]. Junk rows carry sentinel slots and
    are dropped.
"""

import sys

sys.path.insert(0, "/opt/trn_rl_repo")

import ml_dtypes
import numpy as np

import concourse.bass as bass
import concourse.mybir as mybir
import concourse.tile as tile
from concourse import bacc
from concourse.bass_utils import run_bass_kernel_spmd
from concourse.masks import make_identity, make_upper_triangular

F32 = mybir.dt.float32
BF16 = mybir.dt.bfloat16
I32 = mybir.dt.int32
U32 = mybir.dt.uint32

N_CORES = 8
B, S, H, E = 4, 2048, 1024, 8
T = B * S                # 8192 tokens
TPC = T // N_CORES       # 1024 tokens per core slice
TILES = TPC // 128       # 8 token tiles per slice
HC = H // 128            # 8 contraction chunks
BINCAP = 256             # per-(src,dst) bin capacity (observed max 172)
RCAP0 = 1024             # round-0 receiver capacity (slots 0-127; exact bound)
RCAP1 = 256              # round-1 receiver capacity (observed max ~91)
RT0 = RCAP0 // 128       # 8 round-0 GEMM tiles
RT1 = RCAP1 // 128       # 2 round-1 GEMM tiles
NSLOT = N_CORES * BINCAP  # 2048 logical slots
NP = N_CORES * 128       # 1024 rows per payload round
W = 1032                 # bf16 row: 1024 x + gate(f32) + gid(f32)
GCOL = 512               # f32-view column of gate
ICOL = 513               # f32-view column of gid
SENT = 65536.0           # sentinel for empty slots / tails
OOB = 60000.0            # out-of-bounds scatter offset (skipped)
NHALF = 2                # 1024 output dims in 2 x 512 psum halves


def _body(tc, x, rw, rb, ew, eb, gid, erow, iota_r0, iota_r1, slots_r0,
          slots_r1, out_rows, out_gsel0, out_gsel1, out_meta):
    nc = tc.nc
    P = 128
    Exp = mybir.ActivationFunctionType.Exp
    rg = [list(range(N_CORES))]

    dram = tc.alloc_tile_pool(name="dram", bufs=1, space="DRAM")
    pay_comb = dram.tile([NSLOT, W], BF16)
    pay_out = [dram.tile([NP, W], BF16, name=f"pay_out{i}") for i in range(2)]
    meta_in = dram.tile([NSLOT], F32)
    meta_out = dram.tile([NSLOT], F32)
    rflat0 = dram.tile([RCAP0], I32)
    rflat1 = dram.tile([RCAP1], I32)
    dummy_i = dram.tile([8, 4], F32)
    dummy_o = dram.tile([8, 4], F32)

    # First collective fires immediately: pulls every core's rendezvous to
    # kernel start (the alignment barrier ends when the slowest core's first
    # trigger lands).
    nc.gpsimd.collective_compute(
        "AllToAll", mybir.AluOpType.bypass, replica_groups=rg,
        ins=[dummy_i[:].opt()], outs=[dummy_o[:].opt()])

    const = tc.alloc_tile_pool(name="const", bufs=1)
    ident = const.tile([P, P], F32)
    make_identity(nc, ident)
    ones = const.tile([P, P], F32)
    nc.vector.memset(ones[:], 1.0)
    triu = const.tile([P, P], F32)
    make_upper_triangular(nc, triu[:], val=1.0, diag=True)
    identb = const.tile([P, P], BF16)
    nc.vector.tensor_copy(identb[:], ident[:])

    rw_sb = const.tile([P, HC, E], F32)
    nc.sync.dma_start(rw_sb[:], rw.rearrange("(c p) e -> p c e", p=P))
    rb_sb = const.tile([1, E], F32)
    nc.sync.dma_start(rb_sb[:], rb[:])
    rb_rep = const.tile([P, E], F32)
    nc.gpsimd.partition_broadcast(rb_rep[:], rb_sb[:])
    erow_sb = const.tile([1, E], F32)
    nc.sync.dma_start(erow_sb[:], erow[:])
    erow_rep = const.tile([P, E], F32)
    nc.gpsimd.partition_broadcast(erow_rep[:], erow_sb[:])
    gid_sb = const.tile([P, TILES], F32)
    nc.sync.dma_start(gid_sb[:], gid[:])
    iota0_sb = const.tile([16, NSLOT // 16], F32)
    nc.sync.dma_start(iota0_sb[:], iota_r0[:])
    iota1_sb = const.tile([16, NSLOT // 16], F32)
    nc.sync.dma_start(iota1_sb[:], iota_r1[:])
    slots0_sb = const.tile([16, RCAP0 // 16], F32)
    nc.sync.dma_start(slots0_sb[:], slots_r0[:])
    slots1_sb = const.tile([16, RCAP1 // 16], F32)
    nc.sync.dma_start(slots1_sb[:], slots_r1[:])

    w_sb = const.tile([P, HC, H], BF16)
    nc.sync.dma_start(w_sb[:], ew.rearrange("(c p) d -> p c d", p=P))
    eb_sb = const.tile([1, H], F32)
    nc.sync.dma_start(eb_sb[:], eb[:])
    b_rep = const.tile([P, H], F32)
    nc.gpsimd.partition_broadcast(b_rep[:], eb_sb[:])

    # meta_in := sentinel everywhere (slots no scatter writes stay invalid)
    sent16 = const.tile([16, NSLOT // 16], F32)
    nc.vector.memset(sent16[:], SENT)
    nc.sync.dma_start(meta_in[:].rearrange("(p f) -> p f", p=16), sent16[:])

    # ---- Phase A: router + slot assignment + scatter into A2A inputs ----
    ohist = [const.tile([P, E], F32, name=f"ohist{i}") for i in range(TILES)]
    with tc.tile_pool(name="workA", bufs=6) as workA, tc.tile_pool(
        name="psumA", bufs=2, space="PSUM"
    ) as psumA, tc.tile_pool(name="psumL", bufs=2, space="PSUM") as psumL, \
        tc.tile_pool(name="psumP", bufs=2, space="PSUM") as psumP:
        for t in range(TILES):
            xt = workA.tile([P, H], F32, tag="xt")
            nc.sync.dma_start(xt[:], x[t * P : (t + 1) * P, :])
            xT = workA.tile([P, H], F32, tag="xT")
            pt = psumA.tile([P, H], F32, tag="pt")
            for c in range(HC):
                nc.tensor.transpose(
                    pt[:, c * P : (c + 1) * P], xt[:, c * P : (c + 1) * P], ident[:]
                )
            if t % 2 == 0:
                nc.scalar.copy(xT[:], pt[:])
            else:
                nc.vector.tensor_copy(xT[:], pt[:])
            lp = psumL.tile([P, E], F32, tag="lp")
            for c in range(HC):
                nc.tensor.matmul(
                    lp[:],
                    lhsT=xT[:, c * P : (c + 1) * P],
                    rhs=rw_sb[:, c, :],
                    start=(c == 0),
                    stop=(c == HC - 1),
                )
            logits = workA.tile([P, E], F32, tag="logits")
            nc.vector.tensor_tensor(logits[:], lp[:], rb_rep[:], mybir.AluOpType.add)
            negmax = workA.tile([P, 1], F32, tag="negmax")
            nc.vector.reduce_max(
                negmax[:], logits[:], mybir.AxisListType.X, negate=True
            )
            expd = workA.tile([P, E], F32, tag="expd")
            esum = workA.tile([P, 1], F32, tag="esum")
            nc.scalar.activation(
                expd[:], logits[:], Exp, bias=negmax[:], accum_out=esum[:]
            )
            mx8 = workA.tile([P, 8], F32, tag="mx8")
            nc.vector.max(mx8[:], logits[:])
            mi = workA.tile([P, 8], U32, tag="mi")
            nc.vector.max_index(mi[:], mx8[:], logits[:])
            idxf = workA.tile([P, 1], F32, tag="idxf")
            nc.vector.tensor_copy(idxf[:], mi[:, 0:1])

            # one-hot of routed expert; rank via triangular matmul prefix
            nc.vector.tensor_scalar(
                ohist[t][:], erow_rep[:], idxf[:], None,
                op0=mybir.AluOpType.is_equal,
            )
            pfx = psumP.tile([P, E], F32, tag="pfx")
            for a in range(t + 1):
                nc.tensor.matmul(
                    pfx[:],
                    lhsT=ones[:] if a < t else triu[:],
                    rhs=ohist[a][:],
                    start=(a == 0),
                    stop=(a == t),
                )
            ranked = workA.tile([P, E], F32, tag="ranked")
            nc.vector.tensor_tensor(
                ranked[:], pfx[:], ohist[t][:], mybir.AluOpType.mult
            )
            rank = workA.tile([P, 1], F32, tag="rank")
            nc.vector.reduce_sum(rank[:], ranked[:], mybir.AxisListType.X)
            # slot-in-bin (0-based, clamped to bin capacity)
            sb = workA.tile([P, 1], F32, tag="sb")
            nc.vector.tensor_scalar(
                sb[:], rank[:], -1.0, float(BINCAP - 1),
                op0=mybir.AluOpType.add, op1=mybir.AluOpType.min,
            )
            # linear slot for metadata; round-major row for the payload
            slot = workA.tile([P, 1], F32, tag="slot")
            nc.vector.tensor_scalar(
                slot[:], idxf[:], float(BINCAP), sb[:],
                op0=mybir.AluOpType.mult, op1=mybir.AluOpType.add,
            )
            si = workA.tile([P, 1], I32, tag="si")
            nc.vector.tensor_copy(si[:], slot[:])
            tcom = workA.tile([P, 1], F32, tag="tcom")
            nc.vector.tensor_scalar(
                tcom[:], idxf[:], 128.0, sb[:],
                op0=mybir.AluOpType.mult, op1=mybir.AluOpType.add,
            )
            adj = workA.tile([P, 1], F32, tag="adj")
            nc.vector.tensor_scalar(
                adj[:], sb[:], 128.0, 896.0,
                op0=mybir.AluOpType.is_ge, op1=mybir.AluOpType.mult,
            )
            prow = workA.tile([P, 1], F32, tag="prow")
            nc.vector.tensor_tensor(prow[:], tcom[:], adj[:], mybir.AluOpType.add)
            pri = workA.tile([P, 1], I32, tag="pri")
            nc.vector.tensor_copy(pri[:], prow[:])
            nc.gpsimd.indirect_dma_start(
                out=meta_in[:].rearrange("(n one) -> n one", one=1),
                out_offset=bass.IndirectOffsetOnAxis(ap=si[:], axis=0),
                in_=gid_sb[:, t : t + 1],
                in_offset=None,
                bounds_check=NSLOT - 1,
                oob_is_err=False,
            )

            xs = workA.tile([P, W], BF16, tag="xs")
            if t % 2 == 0:
                nc.vector.tensor_copy(xs[:, 0:H], xt[:])
            else:
                nc.scalar.copy(xs[:, 0:H], xt[:])
            xsf = xs[:].bitcast(F32)
            nc.vector.reciprocal(xsf[:, GCOL : GCOL + 1], esum[:])
            nc.vector.tensor_copy(xsf[:, ICOL : ICOL + 1], gid_sb[:, t : t + 1])

            nc.gpsimd.indirect_dma_start(
                out=pay_comb[:],
                out_offset=bass.IndirectOffsetOnAxis(ap=pri[:], axis=0),
                in_=xs[:],
                in_offset=None,
                bounds_check=NSLOT - 1,
                oob_is_err=False,
            )

    # ---- Phase B: metadata A2A, then the two payload rounds ----
    # Trigger order is enforced with tiny dependency-injection writes: each
    # collective's input gets a 16B write sourced from the previous
    # collective's output, so the scheduler cannot reorder the triggers.
    sel = tc.alloc_tile_pool(name="sel", bufs=1)
    d_sb = sel.tile([1, 4], F32)
    nc.sync.dma_start(d_sb[:], dummy_o[0:1, :])
    gate_m = sel.tile([1, 1], F32)
    nc.vector.tensor_scalar(
        gate_m[:], d_sb[0:1, 0:1], 0.0, SENT,
        op0=mybir.AluOpType.mult, op1=mybir.AluOpType.add,
    )
    nc.sync.dma_start(meta_in[NSLOT - 1 : NSLOT].rearrange("(a b) -> a b", b=1),
                      gate_m[:])
    nc.gpsimd.collective_compute(
        "AllToAll", mybir.AluOpType.bypass, replica_groups=rg,
        ins=[meta_in[:].opt()], outs=[meta_out[:].opt()])

    meta16 = sel.tile([16, NSLOT // 16], F32)
    nc.sync.dma_start(meta16[:], meta_out[:].rearrange("(f p) -> p f", p=16))
    nc.sync.dma_start(out_meta[:].rearrange("(f p) -> p f", p=16), meta16[:])

    # row 2047 (bin 7 slot 255) is never occupied: safe to dirty with gates
    gate_p0 = sel.tile([1, 8], BF16)
    nc.vector.tensor_scalar_mul(gate_p0[:], meta16[0:1, 0:8], 0.0)
    nc.sync.dma_start(pay_comb[NSLOT - 1 : NSLOT, 0:8], gate_p0[:])
    nc.gpsimd.collective_compute(
        "AllToAll", mybir.AluOpType.bypass, replica_groups=rg,
        ins=[pay_comb[0:NP, :].opt()], outs=[pay_out[0][:].opt()])
    gate_p1 = sel.tile([1, 8], BF16)
    nc.sync.dma_start(gate_p1[:], pay_out[0][0:1, 0:8])
    nc.sync.dma_start(pay_comb[NSLOT - 1 : NSLOT, 8:16], gate_p1[:])
    nc.gpsimd.collective_compute(
        "AllToAll", mybir.AluOpType.bypass, replica_groups=rg,
        ins=[pay_comb[NP:NSLOT, :].opt()], outs=[pay_out[1][:].opt()])

    # ---- Phase C: receiver compaction from metadata (both rounds) ----
    vmask = sel.tile([16, NSLOT // 16], F32)
    nc.vector.tensor_scalar(
        vmask[:], meta16[:], float(T), None, op0=mybir.AluOpType.is_lt
    )
    ridx = []
    for r, (iota_sb, slots_sb, rcap, rflat, out_gsel) in enumerate([
        (iota0_sb, slots0_sb, RCAP0, rflat0, out_gsel0),
        (iota1_sb, slots1_sb, RCAP1, rflat1, out_gsel1),
    ]):
        val = sel.tile([16, NSLOT // 16], F32)
        nc.vector.tensor_tensor(val[:], iota_sb[:], vmask[:], mybir.AluOpType.mult)
        nc.vector.tensor_scalar_add(val[:], val[:], -1.0)
        rstage = sel.tile([16, rcap // 16], F32)
        rcnt = sel.tile([1, 1], U32)
        nc.gpsimd.sparse_gather(rstage[:], val[:], num_found=rcnt[:])
        rcntf = sel.tile([1, 1], F32)
        nc.vector.tensor_copy(rcntf[:], rcnt[:])
        rcnt16 = sel.tile([16, 1], F32)
        nc.gpsimd.partition_broadcast(rcnt16[:], rcntf[:])
        tailm = sel.tile([16, rcap // 16], F32)
        nc.vector.tensor_scalar(
            tailm[:], slots_sb[:], rcnt16[:], None, op0=mybir.AluOpType.is_lt
        )
        fixed = sel.tile([16, rcap // 16], F32)
        nc.vector.tensor_scalar_add(fixed[:], rstage[:], -SENT)
        nc.vector.tensor_tensor(fixed[:], fixed[:], tailm[:], mybir.AluOpType.mult)
        nc.vector.tensor_scalar_add(fixed[:], fixed[:], SENT)
        ri32 = sel.tile([16, rcap // 16], I32)
        nc.vector.tensor_copy(ri32[:], fixed[:])
        nc.sync.dma_start(rflat[:].rearrange("(f p) -> p f", p=16), ri32[:])
        nc.sync.dma_start(out_gsel[:].rearrange("(f p) -> p f", p=16), ri32[:])
        rx = sel.tile([P, rcap // P], I32, tag=f"ridx{r}")
        nc.sync.dma_start(rx[:], rflat[:].rearrange("(j p) -> p j", p=P))
        ridx.append(rx)

    # ---- Phase D: gather payload rows, expert GEMM, write dense rows ----
    with tc.tile_pool(name="workD", bufs=3) as workD, tc.tile_pool(
        name="gpool", bufs=3
    ) as gpool, tc.tile_pool(name="psumT", bufs=2, space="PSUM") as psumT, \
        tc.tile_pool(name="psumG", bufs=2, space="PSUM") as psumG:
        for j in range(RT0 + RT1):
            r, jj = (0, j) if j < RT0 else (1, j - RT0)
            gath = gpool.tile([P, W], BF16, tag="gath")
            nc.gpsimd.indirect_dma_start(
                out=gath[:],
                out_offset=None,
                in_=pay_out[r][:],
                in_offset=bass.IndirectOffsetOnAxis(
                    ap=ridx[r][:, jj : jj + 1], axis=0
                ),
                bounds_check=NP - 1,
                oob_is_err=False,
            )
            xTg = workD.tile([P, HC, P], BF16, tag="xTg")
            pt = psumT.tile([P, H], BF16, tag="pt")
            for c in range(HC):
                nc.tensor.transpose(
                    pt[:, c * P : (c + 1) * P], gath[:, c * P : (c + 1) * P], identb[:]
                )
            nc.scalar.copy(xTg[:].rearrange("p c d -> p (c d)"), pt[:])
            gate_g = gath[:].bitcast(F32)[:, GCOL : GCOL + 1]
            outj = workD.tile([P, H], F32, tag="outj")
            for h in range(NHALF):
                pg = psumG.tile([P, 512], F32, tag="pg")
                for c in range(HC):
                    nc.tensor.matmul(
                        pg[:],
                        lhsT=xTg[:, c, :],
                        rhs=w_sb[:, c, h * 512 : (h + 1) * 512],
                        start=(c == 0),
                        stop=(c == HC - 1),
                    )
                nc.vector.tensor_tensor(
                    outj[:, h * 512 : (h + 1) * 512],
                    pg[:],
                    b_rep[:, h * 512 : (h + 1) * 512],
                    mybir.AluOpType.add,
                )
                nc.vector.tensor_scalar_mul(
                    outj[:, h * 512 : (h + 1) * 512],
                    outj[:, h * 512 : (h + 1) * 512],
                    gate_g,
                )
            nc.sync.dma_start(out_rows[j * P : (j + 1) * P, :], outj[:])

    sel.release()
    const.release()
    dram.release()


def build_kernel():
    nc = bacc.Bacc(
        "TRN2",
        target_bir_lowering=False,
        debug=False,
        enable_asserts=True,
        num_devices=N_CORES,
    )
    x = nc.dram_tensor("x", [TPC, H], F32, kind="ExternalInput").ap()
    rw = nc.dram_tensor("router_w", [H, E], F32, kind="ExternalInput").ap()
    rb = nc.dram_tensor("router_b", [1, E], F32, kind="ExternalInput").ap()
    ew = nc.dram_tensor("expert_w", [H, H], BF16, kind="ExternalInput").ap()
    eb = nc.dram_tensor("expert_b", [1, H], F32, kind="ExternalInput").ap()
    gid = nc.dram_tensor("gid", [128, TILES], F32, kind="ExternalInput").ap()
    erow = nc.dram_tensor("erow", [1, E], F32, kind="ExternalInput").ap()
    iota_r0 = nc.dram_tensor(
        "iota_r0", [16, NSLOT // 16], F32, kind="ExternalInput"
    ).ap()
    iota_r1 = nc.dram_tensor(
        "iota_r1", [16, NSLOT // 16], F32, kind="ExternalInput"
    ).ap()
    slots_r0 = nc.dram_tensor(
        "slots_r0", [16, RCAP0 // 16], F32, kind="ExternalInput"
    ).ap()
    slots_r1 = nc.dram_tensor(
        "slots_r1", [16, RCAP1 // 16], F32, kind="ExternalInput"
    ).ap()
    out_rows = nc.dram_tensor(
        "out_rows", [RCAP0 + RCAP1, H], F32, kind="ExternalOutput"
    ).ap()
    out_gsel0 = nc.dram_tensor("out_gsel0", [RCAP0], I32, kind="ExternalOutput").ap()
    out_gsel1 = nc.dram_tensor("out_gsel1", [RCAP1], I32, kind="ExternalOutput").ap()
    out_meta = nc.dram_tensor("out_meta", [NSLOT], F32, kind="ExternalOutput").ap()

    with tile.TileContext(nc) as tc:
        _body(tc, x, rw, rb, ew, eb, gid, erow, iota_r0, iota_r1, slots_r0,
              slots_r1, out_rows, out_gsel0, out_gsel1, out_meta)
    nc.compile()
    return nc


_CACHE = {}


def _wrap16(vals):
    """Values laid out so element k sits at [k % 16, k // 16]."""
    a = np.asarray(vals, dtype=np.float32)
    return a.reshape(-1, 16).T.copy()


def kernel(x, router_w, router_b, expert_w, expert_b, **run_kwargs):
    x = np.ascontiguousarray(np.asarray(x, dtype=np.float32))
    router_w = np.ascontiguousarray(np.asarray(router_w, dtype=np.float32))
    router_b = np.ascontiguousarray(np.asarray(router_b, dtype=np.float32))
    expert_w = np.ascontiguousarray(np.asarray(expert_w, dtype=np.float32))
    expert_b = np.ascontiguousarray(np.asarray(expert_b, dtype=np.float32))

    hs = x.reshape(T, H)
    # per-slot values for the receiver selects: round-local payload row + 1
    # where slot g = e*256 + sb belongs to the round, else 0
    g = np.arange(NSLOT)
    e, sb = g // BINCAP, g % BINCAP
    i0 = np.where(sb < 128, e * 128 + sb + 1, 0).astype(np.float32)
    i1 = np.where(sb >= 128, e * 128 + (sb - 128) + 1, 0).astype(np.float32)
    iota_r0 = _wrap16(i0)
    iota_r1 = _wrap16(i1)
    slots_r0 = _wrap16(np.arange(RCAP0, dtype=np.float32))
    slots_r1 = _wrap16(np.arange(RCAP1, dtype=np.float32))
    erow = np.arange(E, dtype=np.float32).reshape(1, E)

    if "nc" not in _CACHE:
        _CACHE["nc"] = build_kernel()
    nc = _CACHE["nc"]

    in_maps = []
    for c in range(N_CORES):
        gid = (
            c * TPC
            + np.arange(TILES)[None, :] * 128
            + np.arange(128)[:, None]
        ).astype(np.float32)
        in_maps.append(
            {
                "x": hs[c * TPC : (c + 1) * TPC],
                "router_w": router_w,
                "router_b": router_b.reshape(1, E),
                "expert_w": expert_w[c].astype(ml_dtypes.bfloat16),
                "expert_b": expert_b[c].reshape(1, H),
                "gid": gid,
                "erow": erow,
                "iota_r0": iota_r0,
                "iota_r1": iota_r1,
                "slots_r0": slots_r0,
                "slots_r1": slots_r1,
            }
        )

    res = run_bass_kernel_spmd(nc, in_maps, core_ids=list(range(N_CORES)), **run_kwargs)
    full = np.zeros((T, H), dtype=np.float32)
    for r in res.results:
        meta = r["out_meta"]
        rows = r["out_rows"]
        for rnd, (gsel, base, off) in enumerate(
            [(r["out_gsel0"], 0, 0), (r["out_gsel1"], RCAP0, 128)]
        ):
            valid = (gsel >= 0) & (gsel < NP)
            rowid = np.nonzero(valid)[0]
            gv = gsel[valid].astype(np.int64)
            e, sb = gv // 128, gv % 128 + off
            gids = meta[e * BINCAP + sb].astype(np.int64)
            inner = (gids >= 0) & (gids < T)
            full[gids[inner]] = rows[base + rowid[inner]]
    out = full.reshape(B, S, H)
    if run_kwargs:
        return out, res
    return out
